# revision 1
# baseline (speedup 1.0000x reference)
"""GCN (2-layer GraphConv, norm='both') on 8 Trainium2 NeuronCores.

Strategy (graph/data parallel, per the node-partition sharding):
  - Nodes are partitioned into 8 contiguous shards; each core owns its shard's
    in-edges (edges grouped by dst).  Edges are sorted by dst on the host and
    chunked into 128-dst-node groups.
  - Dispatch 1 (conv0): each core aggregates its in-edges' 3-wide source
    features (edge payloads laid out host-side, halo-exchange style), applies
    D_in^-1/2, W0, b0, leaky-relu, and the next layer's D_out^-1/2 pre-scale,
    producing its shard of hs = leaky_relu(conv0) * out_deg^-1/2.
  - Host concatenates the 8 hs shards (pure layout, no math).
  - Dispatch 2 (conv1): each core gathers hs[src] for its in-edges with the
    SWDGE dma_gather (512B rows from HBM), segment-sums per 128-dst chunk via
    one-hot-indicator matmuls accumulating in PSUM, applies D_in^-1/2, W1, b1.
  - Host concatenates the 8 output shards.

All O(E*D) / O(N*D) compute and memory traffic runs on-device; the host does
index manipulation (sort/pad/relabel), normalization constants, and the
3-float-per-edge conv0 payload layout.
"""

import os
from contextlib import ExitStack

import numpy as np

import concourse.bass as bass
import concourse.tile as tile
from concourse import bacc, mybir
from concourse._compat import with_exitstack
from concourse.alu_op_type import AluOpType
from concourse.bass_utils import run_bass_kernel_spmd

F32 = mybir.dt.float32
I16 = mybir.dt.int16

NC_CORES = 8
D = 128          # feature dim of both conv layers
NEG_SLOPE = 0.01

# filled by kernel() for test harnesses to inspect
LAST_EXEC_TIMES_NS: list = []
LAST_RESULTS: list = []


# --------------------------------------------------------------------------
# host-side prep
# --------------------------------------------------------------------------

def _wrap_idx(idx: np.ndarray) -> np.ndarray:
    """dma_gather index layout: position i lives at [i % 16, i // 16] of a
    16-row wrap, replicated 8x (one copy per Q7 core) -> [128, n/16] int16."""
    n = idx.shape[0]
    assert n % 16 == 0
    return np.tile(idx.astype(np.int16).reshape(n // 16, 16).T, (8, 1))


def _prep(src, dst, weight, significance, emb, W0, b0, W1, b1):
    n = weight.shape[0]
    e = src.shape[0]
    npc = n // NC_CORES                    # nodes per core (owned shard)
    assert npc * NC_CORES == n
    n_chunks = (npc + 127) // 128          # 128-dst-node chunks per core
    half = (n + 1) // 2                    # src-id bucket split for int16 idx
    assert half <= 32767 and n - half <= 32767

    src = np.asarray(src).astype(np.int64)
    dst = np.asarray(dst).astype(np.int64)

    out_deg = np.bincount(src, minlength=n).astype(np.float64)
    in_deg = np.bincount(dst, minlength=n).astype(np.float64)
    od = (1.0 / np.sqrt(np.clip(out_deg, 1.0, None))).astype(np.float32)
    ri = (1.0 / np.sqrt(np.clip(in_deg, 1.0, None))).astype(np.float32)

    # conv0 per-edge source payload (halo-exchange layout):
    #   m_e = od[src] * [w[src], emb[sig[src],0], emb[sig[src],1], 0]
    emb_rows = np.asarray(emb, np.float32)[np.asarray(significance).astype(np.int64)]
    feats = np.concatenate(
        [np.asarray(weight, np.float32)[:, None], emb_rows], axis=1
    ) * od[:, None]                                        # [n, 3]

    order = np.argsort(dst, kind="stable")
    s_src, s_dst = src[order], dst[order]

    # per-(core, chunk) and per-(core, chunk, bucket) edge slices
    core_of = s_dst // npc
    loc = s_dst - core_of * npc
    chunk_of = loc // 128
    e_starts = np.searchsorted(core_of * n_chunks + chunk_of,
                               np.arange(NC_CORES * n_chunks + 1))

    # bucket within chunks by src half (stable -> still dst sorted inside)
    cnt0 = np.zeros((NC_CORES, n_chunks), np.int64)
    cnt1 = np.zeros((NC_CORES, n_chunks), np.int64)
    for c in range(NC_CORES):
        for k in range(n_chunks):
            s0, s1 = e_starts[c * n_chunks + k], e_starts[c * n_chunks + k + 1]
            lo = int(np.count_nonzero(s_src[s0:s1] < half))
            cnt0[c, k], cnt1[c, k] = lo, (s1 - s0) - lo

    t0 = max(1, int(-(-int((cnt0 + cnt1).max()) // 128)))   # conv0 tiles/chunk
    t1 = max(1, int(-(-int(max(cnt0.max(), cnt1.max())) // 128)))  # per bucket

    n0 = t0 * 128
    n1 = t1 * 128

    x0h = np.zeros((NC_CORES, 128, n_chunks * t0 * 4), np.float32)
    dv0 = np.full((NC_CORES, 128, n_chunks * t0), -1.0, np.float32)
    idxh = np.zeros((NC_CORES, 128, n_chunks * 2 * (n1 // 16)), np.int16)
    dv1 = np.full((NC_CORES, 128, n_chunks * 2 * t1), -1.0, np.float32)

    for c in range(NC_CORES):
        for k in range(n_chunks):
            s0, s1 = e_starts[c * n_chunks + k], e_starts[c * n_chunks + k + 1]
            es, ed = s_src[s0:s1], s_dst[s0:s1]
            dloc = (ed - c * npc - k * 128).astype(np.float32)

            # conv0: all edges of the chunk, padded to n0
            pay = np.zeros((n0, 4), np.float32)
            pay[: s1 - s0, :3] = feats[es]
            x0h[c, :, k * t0 * 4:(k + 1) * t0 * 4] = (
                pay.reshape(t0, 128, 4).transpose(1, 0, 2).reshape(128, t0 * 4)
            )
            dvc = np.full(n0, -1.0, np.float32)
            dvc[: s1 - s0] = dloc
            dv0[c, :, k * t0:(k + 1) * t0] = dvc.reshape(t0, 128).T

            # conv1: bucket by src half, padded to n1 each
            m0 = es < half
            for b, mask in ((0, m0), (1, ~m0)):
                sb = es[mask] - (0 if b == 0 else half)
                db = dloc[mask]
                pidx = np.zeros(n1, np.int64)
                pidx[: sb.shape[0]] = sb
                w16 = n1 // 16
                idxh[c, :, (k * 2 + b) * w16:(k * 2 + b + 1) * w16] = _wrap_idx(pidx)
                dvb = np.full(n1, -1.0, np.float32)
                dvb[: db.shape[0]] = db
                dv1[c, :, (k * 2 + b) * t1:(k * 2 + b + 1) * t1] = (
                    dvb.reshape(t1, 128).T
                )

    # per-core [128, n_chunks] normalization tables (pad rows -> 1.0)
    def _pc(v):
        out = np.ones((NC_CORES, 128, n_chunks), np.float32)
        for c in range(NC_CORES):
            vv = np.ones(n_chunks * 128, np.float32)
            vv[:npc] = v[c * npc:(c + 1) * npc]
            out[c] = vv.reshape(n_chunks, 128).T
        return out

    consts = {
        "iota_bc": np.tile(np.arange(128, dtype=np.float32)[None, :], (128, 1)),
        "b0_bc": np.tile(np.asarray(b0, np.float32)[None, :], (128, 1)),
        "b1_bc": np.tile(np.asarray(b1, np.float32)[None, :], (128, 1)),
        "W0p": np.concatenate(
            [np.asarray(W0, np.float32), np.zeros((1, D), np.float32)], axis=0
        ),
        "W1": np.asarray(W1, np.float32),
    }
    return dict(
        n=n, e=e, npc=npc, n_chunks=n_chunks, half=half, t0=t0, t1=t1,
        od_pc=_pc(od), ri_pc=_pc(ri), x0h=x0h, dv0=dv0, idxh=idxh, dv1=dv1,
        consts=consts,
    )


# --------------------------------------------------------------------------
# device programs
# --------------------------------------------------------------------------

def _new_nc():
    return bacc.Bacc("TRN2", target_bir_lowering=False, debug=False,
                     num_devices=NC_CORES)


@with_exitstack
def _conv0_body(ctx: ExitStack, tc, aps, n_chunks, t0):
    nc = tc.nc
    cpool = ctx.enter_context(tc.tile_pool(name="consts", bufs=1))
    pool = ctx.enter_context(tc.tile_pool(name="work", bufs=3))
    epool = ctx.enter_context(tc.tile_pool(name="epi", bufs=3))
    ps_a = ctx.enter_context(tc.tile_pool(name="ps_a", bufs=2, space="PSUM"))
    ps_g = ctx.enter_context(tc.tile_pool(name="ps_g", bufs=2, space="PSUM"))

    iota_sb = cpool.tile([128, 128], F32)
    nc.sync.dma_start(iota_sb[:], aps["iota_bc"][:])
    b0_sb = cpool.tile([128, 128], F32)
    nc.sync.dma_start(b0_sb[:], aps["b0_bc"][:])
    w0_sb = cpool.tile([4, D], F32)
    nc.sync.dma_start(w0_sb[:], aps["W0p"][:])
    dv0_sb = cpool.tile([128, n_chunks * t0], F32)
    nc.sync.dma_start(dv0_sb[:], aps["dv0"][:])
    od_sb = cpool.tile([128, n_chunks], F32)
    nc.sync.dma_start(od_sb[:], aps["od_pc"][:])
    ri_sb = cpool.tile([128, n_chunks], F32)
    nc.sync.dma_start(ri_sb[:], aps["ri_pc"][:])

    x0_sb = cpool.tile([128, n_chunks * t0 * 4], F32)
    nc.sync.dma_start(x0_sb[:], aps["x0h"][:])
    hs_d = aps["hs"]        # [n_chunks * 128, D] output

    for k in range(n_chunks):
        x0_k = x0_sb[:, k * t0 * 4:(k + 1) * t0 * 4]
        ind_sb = pool.tile([128, t0 * 128], F32, tag="ind")
        nc.vector.tensor_tensor(
            ind_sb[:].rearrange("p (t j) -> p t j", j=128),
            dv0_sb[:, k * t0:(k + 1) * t0].unsqueeze(2).broadcast_to([128, t0, 128]),
            iota_sb[:].unsqueeze(1).broadcast_to([128, t0, 128]),
            AluOpType.is_equal,
        )
        agg_ps = ps_a.tile([4, 128], F32, tag="agg")
        for t in range(t0):
            nc.tensor.matmul(
                agg_ps[:],
                lhsT=x0_k[:, bass.ts(t, 4)],
                rhs=ind_sb[:, bass.ts(t, 128)],
                start=(t == 0),
                stop=(t == t0 - 1),
            )
        agg_sb = epool.tile([4, 128], F32, tag="aggsb")
        nc.vector.tensor_copy(agg_sb[:], agg_ps[:])

        g_ps = ps_g.tile([128, D], F32, tag="g")
        nc.tensor.matmul(g_ps[:], lhsT=agg_sb[:], rhs=w0_sb[:], start=True, stop=True)

        t_sb = epool.tile([128, D], F32, tag="t")
        nc.vector.scalar_tensor_tensor(
            t_sb[:], g_ps[:], ri_sb[:, k:k + 1], b0_sb[:],
            AluOpType.mult, AluOpType.add,
        )
        u_sb = epool.tile([128, D], F32, tag="u")
        nc.scalar.activation(u_sb[:], t_sb[:], mybir.ActivationFunctionType.Copy,
                             scale=od_sb[:, k:k + 1])
        hs_sb = epool.tile([128, D], F32, tag="hs")
        nc.vector.scalar_tensor_tensor(
            hs_sb[:], u_sb[:], float(NEG_SLOPE), u_sb[:],
            AluOpType.mult, AluOpType.max,
        )
        nc.sync.dma_start(hs_d[k * 128:(k + 1) * 128, :], hs_sb[:])


@with_exitstack
def _conv1_body(ctx: ExitStack, tc, aps, n_chunks, t1, half, n):
    nc = tc.nc
    cpool = ctx.enter_context(tc.tile_pool(name="consts", bufs=1))
    pool = ctx.enter_context(tc.tile_pool(name="work", bufs=6))
    epool = ctx.enter_context(tc.tile_pool(name="epi", bufs=4))
    ps_a = ctx.enter_context(tc.tile_pool(name="ps_a", bufs=4, space="PSUM"))
    ps_o = ctx.enter_context(tc.tile_pool(name="ps_o", bufs=2, space="PSUM"))

    iota_sb = cpool.tile([128, 128], F32)
    nc.sync.dma_start(iota_sb[:], aps["iota_bc"][:])
    b1_sb = cpool.tile([128, 128], F32)
    nc.sync.dma_start(b1_sb[:], aps["b1_bc"][:])
    w1_sb = cpool.tile([D, D], F32)
    nc.sync.dma_start(w1_sb[:], aps["W1"][:])
    dv1_sb = cpool.tile([128, n_chunks * 2 * t1], F32)
    nc.sync.dma_start(dv1_sb[:], aps["dv1"][:])
    ri_sb = cpool.tile([128, n_chunks], F32)
    nc.sync.dma_start(ri_sb[:], aps["ri_pc"][:])

    hs_d = aps["hs"]        # [n, D] full pre-scaled features
    out_d = aps["out"]      # [n_chunks * 128, D]
    n1 = t1 * 128
    w16 = n1 // 16
    idx_sb = cpool.tile([128, n_chunks * 2 * w16], I16)
    nc.sync.dma_start(idx_sb[:], aps["idxh"][:])

    for k in range(n_chunks):
        xb = []
        for b in range(2):
            x_sb = pool.tile([128, t1 * D], F32, tag="x")
            src_rows = hs_d[0:half, :] if b == 0 else hs_d[half:n, :]
            nc.gpsimd.dma_gather(
                out_ap=x_sb[:].rearrange("p (t f) -> p t f", f=D),
                in_ap=src_rows,
                idxs_ap=idx_sb[:, (k * 2 + b) * w16:(k * 2 + b + 1) * w16],
                num_idxs=n1,
                num_idxs_reg=n1,
                elem_size=D,
                # >64 descriptors per SDMA engine must not share one packet
                single_packet=(t1 * 128 // 16 <= 63),
            )
            xb.append(x_sb)

        ind_sb = pool.tile([128, 2 * t1 * 128], F32, tag="ind")
        nc.vector.tensor_tensor(
            ind_sb[:].rearrange("p (t j) -> p t j", j=128),
            dv1_sb[:, k * 2 * t1:(k + 1) * 2 * t1]
                .unsqueeze(2).broadcast_to([128, 2 * t1, 128]),
            iota_sb[:].unsqueeze(1).broadcast_to([128, 2 * t1, 128]),
            AluOpType.is_equal,
        )
        agg_ps = ps_a.tile([D, 128], F32, tag="agg")
        for b in range(2):
            for t in range(t1):
                nc.tensor.matmul(
                    agg_ps[:],
                    lhsT=xb[b][:, bass.ts(t, D)],
                    rhs=ind_sb[:, bass.ts(b * t1 + t, 128)],
                    start=(b == 0 and t == 0),
                    stop=(b == 1 and t == t1 - 1),
                )
        agg_sb = epool.tile([D, 128], F32, tag="aggsb")
        nc.vector.tensor_copy(agg_sb[:], agg_ps[:])

        o_ps = ps_o.tile([128, D], F32, tag="o")
        nc.tensor.matmul(o_ps[:], lhsT=agg_sb[:], rhs=w1_sb[:], start=True, stop=True)

        out_sb = epool.tile([128, D], F32, tag="outsb")
        nc.vector.scalar_tensor_tensor(
            out_sb[:], o_ps[:], ri_sb[:, k:k + 1], b1_sb[:],
            AluOpType.mult, AluOpType.add,
        )
        nc.sync.dma_start(out_d[k * 128:(k + 1) * 128, :], out_sb[:])


def tensor_specs0(p):
    n_chunks, t0 = p["n_chunks"], p["t0"]
    return {
        "iota_bc": ((128, 128), F32, "ExternalInput"),
        "b0_bc": ((128, 128), F32, "ExternalInput"),
        "W0p": ((4, D), F32, "ExternalInput"),
        "dv0": ((128, n_chunks * t0), F32, "ExternalInput"),
        "od_pc": ((128, n_chunks), F32, "ExternalInput"),
        "ri_pc": ((128, n_chunks), F32, "ExternalInput"),
        "x0h": ((128, n_chunks * t0 * 4), F32, "ExternalInput"),
        "hs": ((n_chunks * 128, D), F32, "ExternalOutput"),
    }


def tensor_specs1(p):
    n, n_chunks, t1 = p["n"], p["n_chunks"], p["t1"]
    return {
        "iota_bc": ((128, 128), F32, "ExternalInput"),
        "b1_bc": ((128, 128), F32, "ExternalInput"),
        "W1": ((D, D), F32, "ExternalInput"),
        "dv1": ((128, n_chunks * 2 * t1), F32, "ExternalInput"),
        "ri_pc": ((128, n_chunks), F32, "ExternalInput"),
        "idxh": ((128, n_chunks * 2 * (t1 * 128 // 16)), I16, "ExternalInput"),
        "hs": ((n, D), F32, "ExternalInput"),
        "out": ((n_chunks * 128, D), F32, "ExternalOutput"),
    }


def in_maps0(p):
    c = p["consts"]
    return [
        {"iota_bc": c["iota_bc"], "b0_bc": c["b0_bc"], "W0p": c["W0p"],
         "dv0": p["dv0"][i], "od_pc": p["od_pc"][i], "ri_pc": p["ri_pc"][i],
         "x0h": p["x0h"][i]}
        for i in range(NC_CORES)
    ]


def in_maps1(p, hs_full):
    c = p["consts"]
    return [
        {"iota_bc": c["iota_bc"], "b1_bc": c["b1_bc"], "W1": c["W1"],
         "dv1": p["dv1"][i], "ri_pc": p["ri_pc"][i], "idxh": p["idxh"][i],
         "hs": hs_full}
        for i in range(NC_CORES)
    ]


def _build(body, tensors, **kw):
    """tensors: name -> (shape, dtype, kind). Returns (nc, name->AP)."""
    nc = _new_nc()
    aps = {
        name: nc.dram_tensor(name, list(shape), dtype, kind=kind).ap()
        for name, (shape, dtype, kind) in tensors.items()
    }
    with tile.TileContext(nc) as tc:
        body(tc, aps, **kw)
    nc.compile()
    return nc


# --------------------------------------------------------------------------
# entry point
# --------------------------------------------------------------------------

def kernel(src, dst, weight, significance, emb, W0, b0, W1, b1):
    global LAST_EXEC_TIMES_NS, LAST_RESULTS
    LAST_EXEC_TIMES_NS = []
    LAST_RESULTS = []
    trace = bool(os.environ.get("BASS_TRACE"))

    p = _prep(src, dst, weight, significance, emb, W0, b0, W1, b1)
    n, npc, n_chunks, t0, t1 = p["n"], p["npc"], p["n_chunks"], p["t0"], p["t1"]
    half = p["half"]
    npad = n_chunks * 128
    c = p["consts"]

    nc0 = _build(_conv0_body, tensor_specs0(p), n_chunks=n_chunks, t0=t0)
    res0 = run_bass_kernel_spmd(nc0, in_maps0(p), core_ids=list(range(NC_CORES)),
                                trace=trace)
    LAST_RESULTS.append(res0)
    LAST_EXEC_TIMES_NS.append(res0.exec_time_ns)
    hs_full = np.concatenate(
        [res0.results[i]["hs"][:npc] for i in range(NC_CORES)], axis=0
    )
    assert hs_full.shape == (n, D)

    nc1 = _build(_conv1_body, tensor_specs1(p), n_chunks=n_chunks, t1=t1,
                 half=half, n=n)
    res1 = run_bass_kernel_spmd(nc1, in_maps1(p, hs_full),
                                core_ids=list(range(NC_CORES)), trace=trace)
    LAST_RESULTS.append(res1)
    LAST_EXEC_TIMES_NS.append(res1.exec_time_ns)

    out = np.concatenate(
        [res1.results[i]["out"][:npc] for i in range(NC_CORES)], axis=0
    )
    assert out.shape == (n, D)
    return out.astype(np.float32)

